# revision 21
# baseline (speedup 1.0000x reference)
"""EquiEncoder (gnn_message_passing) Trainium2 kernel, 8-core SPMD.

Computes (H, h) of the reference. Key simplification: the reference's vector
channel (v / V / dv_ij / dv_iI) never feeds the returned outputs, and only the
first 128 of the 384 MLP output columns matter (s0 / c0).

Sharding: beads (CG) partitioned into 8 contiguous blocks; atoms co-located
with their bead; directed edges assigned to the receiver atom's core and
grouped into 128-receiver "dst tiles". Per layer: local MLP -> phi rows ->
AllGather -> per-edge dma_gather of phi rows -> msg = phi_src * w_s ->
segment-sum via PE matmul against on-chip 0/1 selection matrices ->
h update. H accumulated in one long-lived PSUM via scatter-mean matmuls.
"""

import math
import os
import numpy as np

import concourse.bass as bass
import concourse.bacc as bacc
import concourse.mybir as mybir
import concourse.tile as tile
from concourse import bass_utils

# ---------------- problem constants (hardcoded per contract) ----------------
N_ATOMS = 12000
N_CG = 1500
N_NBR = 64000
FEAT = 128
N_RBF = 16
N_CONV = 2
CUTOFF = 5.0
CG_CUTOFF = 12.5

NCORES = 8
APAD = 1664            # per-core atom capacity (13 tiles of 128)
TA = APAD // 128       # 13 atom tiles per core
BPAD = 192             # per-core bead capacity
NPAD = NCORES * APAD   # 13312 global (padded) atom id space

F32 = mybir.dt.float32
I16 = mybir.dt.int16

_CACHE = {}
LAST_RESULT = None  # BassKernelResults of the most recent run (for test harness)
_LAST_NC = None
_LAST_INMAPS = None


def rerun(n=1):
    """Re-execute the last-built kernel (for wall-clock timing)."""
    import time as _t
    ts = []
    for _ in range(n):
        t0 = _t.time()
        bass_utils.run_bass_kernel_spmd(
            _LAST_NC, _LAST_INMAPS, core_ids=list(range(NCORES)))
        ts.append(_t.time() - t0)
    return ts


# ================================ host prep ================================

def _host_prep(z, xyz, cg_z, cg_xyz, mapping, nbr_list,
               atom_emb, res_emb,
               msg_W1, msg_b1, msg_W2, msg_b2, msg_Wd, msg_bd,
               cg_W1, cg_b1, cg_W2, cg_b2, cg_Wd, cg_bd):
    """Pure index manipulation / sharding. Returns (cfg, in_maps, asm)."""
    z = np.asarray(z).astype(np.int64)
    cg_z = np.asarray(cg_z).astype(np.int64)
    mapping = np.asarray(mapping).astype(np.int64)
    nbr = np.asarray(nbr_list).astype(np.int64)
    xyz = np.asarray(xyz, dtype=np.float32)
    cg_xyz = np.asarray(cg_xyz, dtype=np.float32)

    # ---- bead -> core (contiguous blocks), atom -> core of its bead
    per = [N_CG // NCORES + (1 if c < N_CG % NCORES else 0) for c in range(NCORES)]
    bbounds = np.concatenate([[0], np.cumsum(per)])
    bead_core = np.searchsorted(bbounds, np.arange(N_CG), side="right") - 1
    bead_local = np.arange(N_CG) - bbounds[bead_core]
    atom_core = bead_core[mapping]

    # stable order: by core, then bead, preserving original order within
    order = np.lexsort((np.arange(N_ATOMS), mapping, atom_core))
    newid = np.full(N_ATOMS, -1, dtype=np.int64)
    atoms_per_core = []
    for c in range(NCORES):
        a_c = order[atom_core[order] == c]
        assert len(a_c) <= APAD, f"core {c} atoms {len(a_c)} > {APAD}"
        newid[a_c] = c * APAD + np.arange(len(a_c))
        atoms_per_core.append(a_c)

    # bead counts for scatter_mean (clamped to 1 like the reference)
    cnt = np.maximum(np.bincount(mapping, minlength=N_CG), 1).astype(np.float32)

    # ---- directed edges, assigned to receiver's core, grouped by dst tile
    recv = np.concatenate([nbr[:, 0], nbr[:, 1]])
    srcj = np.concatenate([nbr[:, 1], nbr[:, 0]])
    rn = newid[recv]
    sn = newid[srcj]
    e_core = rn // APAD
    e_loc = rn % APAD
    e_tile = e_loc // 128
    e_dloc = e_loc % 128

    # per (core, tile) edge index lists
    counts = np.zeros((NCORES, TA), dtype=np.int64)
    lists = [[None] * TA for _ in range(NCORES)]
    key = e_core * TA + e_tile
    sort_by_key = np.argsort(key, kind="stable")
    ksorted = key[sort_by_key]
    starts = np.searchsorted(ksorted, np.arange(NCORES * TA))
    ends = np.searchsorted(ksorted, np.arange(NCORES * TA), side="right")
    for c in range(NCORES):
        for t in range(TA):
            k = c * TA + t
            idxs = sort_by_key[starts[k]:ends[k]]
            lists[c][t] = idxs
            counts[c, t] = len(idxs)

    # static per-tile subchunk counts, shared across cores (SPMD)
    C_t = [max(1, int(math.ceil(counts[:, t].max() / 128))) for t in range(TA)]
    SUMC = sum(C_t)
    offs = np.concatenate([[0], np.cumsum(C_t)])  # subchunk offset per tile

    SC_ALL = SUMC + TA        # + one rbf subchunk per atom tile
    SC8 = ((SC_ALL + 7) // 8) * 8

    cfg = dict(C_t=tuple(C_t), SUMC=SUMC, SC_ALL=SC_ALL, SC8=SC8)

    # ---- per-core edge arrays
    esrc_w = np.zeros((NCORES, 128, SUMC * 8), dtype=np.int16)
    edst = np.full((NCORES, 128, SUMC), -1.0, dtype=np.float32)
    exyz = np.zeros((NCORES, 128, SUMC, 6), dtype=np.float32)
    exyz[..., 3] = 1.0  # pad edges: A=(0,0,0), B=(1,0,0) -> d=1
    for c in range(NCORES):
        for t in range(TA):
            idxs = lists[c][t]
            n = len(idxs)
            cap = C_t[t] * 128
            src_ids = np.full(cap, c * APAD, dtype=np.int64)
            dlocs = np.full(cap, -1.0, dtype=np.float32)
            xa = np.zeros((cap, 3), dtype=np.float32)
            xb = np.zeros((cap, 3), dtype=np.float32)
            xb[:, 0] = 1.0
            if n:
                src_ids[:n] = sn[idxs]
                dlocs[:n] = e_dloc[idxs].astype(np.float32)
                xa[:n] = xyz[srcj[idxs]]
                xb[:n] = xyz[recv[idxs]]
            # gather idx wrapping: [16, cap/16] replicated to 128 partitions
            w = np.tile(src_ids.astype(np.int16).reshape(cap // 16, 16).T, (8, 1))
            esrc_w[c, :, offs[t] * 8:(offs[t] + C_t[t]) * 8] = w
            sl = slice(offs[t], offs[t] + C_t[t])
            edst[c, :, sl] = dlocs.reshape(C_t[t], 128).T
            exyz[c, :, sl, 0:3] = xa.reshape(C_t[t], 128, 3).transpose(1, 0, 2)
            exyz[c, :, sl, 3:6] = xb.reshape(C_t[t], 128, 3).transpose(1, 0, 2)

    # ---- per-core atom arrays
    axyz = np.zeros((NCORES, 128, TA, 6), dtype=np.float32)
    axyz[..., 3] = 1.0
    oh75 = np.zeros((NCORES, 75, APAD), dtype=np.float32)
    mmean = np.zeros((NCORES, 128, TA, BPAD), dtype=np.float32)
    for c in range(NCORES):
        a_c = atoms_per_core[c]
        n = len(a_c)
        loc = np.arange(n)
        p, t = loc % 128, loc // 128
        axyz[c, p, t, 0:3] = xyz[a_c]
        axyz[c, p, t, 3:6] = cg_xyz[mapping[a_c]]
        oh75[c, z[a_c], loc] = 1.0
        oh75[c, 50 + cg_z[mapping[a_c]], loc] = 1.0
        lb = bead_local[mapping[a_c]]
        mmean[c, p, t, lb] = 1.0 / cnt[mapping[a_c]]

    # ---- replicated constants
    E75 = np.zeros((75, 128), dtype=np.float32)
    E75[0:50, 0:64] = np.asarray(atom_emb, dtype=np.float32)
    E75[50:75, 64:128] = np.asarray(res_emb, dtype=np.float32)
    # 32-slot rbf patterns: [n*pi/cut for n=1..16] + zeros
    ncce = np.zeros((128, 32), dtype=np.float32)
    ncce[:, 0:16] = np.arange(1, 17, dtype=np.float32) * math.pi / CUTOFF
    ncca = np.zeros((128, 32), dtype=np.float32)
    ncca[:, 0:16] = np.arange(1, 17, dtype=np.float32) * math.pi / CG_CUTOFF
    add32 = np.zeros((128, 32), dtype=np.float32)
    add32[:, 16] = 1.0
    iota = np.tile(np.arange(128, dtype=np.float32), (128, 1))
    halfpi = np.full((128, 1), math.pi / 2, dtype=np.float32)
    ident = np.eye(128, dtype=np.float32)

    def aug_w(Wd, bd):
        # [32, 128]: rows 0:16 Wd[:, :128], row 16 bd[:128], rest zero
        w = np.zeros((32, 128), dtype=np.float32)
        w[0:16] = np.asarray(Wd, dtype=np.float32)[:, :FEAT]
        w[16] = np.asarray(bd, dtype=np.float32)[:FEAT]
        return w

    weights = {}
    for l in range(N_CONV):
        weights[f"mW1_{l}"] = np.asarray(msg_W1[l], dtype=np.float32)
        weights[f"mb1_{l}"] = np.asarray(msg_b1[l], dtype=np.float32).reshape(128, 1)
        weights[f"mW2_{l}"] = np.asarray(msg_W2[l], dtype=np.float32)[:, :FEAT].copy()
        weights[f"mb2_{l}"] = np.tile(np.asarray(msg_b2[l], dtype=np.float32)[None, :FEAT],
                                      (128, 1))
        weights[f"mWd_{l}"] = aug_w(msg_Wd[l], msg_bd[l])
        weights[f"cW1_{l}"] = np.asarray(cg_W1[l], dtype=np.float32)
        weights[f"cb1_{l}"] = np.asarray(cg_b1[l], dtype=np.float32).reshape(128, 1)
        weights[f"cW2_{l}"] = np.asarray(cg_W2[l], dtype=np.float32)[:, :FEAT].copy()
        weights[f"cb2_{l}"] = np.tile(np.asarray(cg_b2[l], dtype=np.float32)[None, :FEAT],
                                      (128, 1))
        weights[f"cWd_{l}"] = aug_w(cg_Wd[l], cg_bd[l])

    in_maps = []
    for c in range(NCORES):
        m = dict(weights)
        m["oh75"] = oh75[c]
        m["E75"] = E75
        m["mmean"] = mmean[c].reshape(128, TA * BPAD)
        m["esrc"] = esrc_w[c]
        m["edst"] = edst[c]
        m["exyz"] = exyz[c].reshape(128, SUMC * 6)
        m["axyz"] = axyz[c].reshape(128, TA * 6)
        m["ncce"] = ncce
        m["ncca"] = ncca
        m["add32"] = add32
        m["iota"] = iota
        m["halfpi"] = halfpi
        m["ident"] = ident
        in_maps.append(m)

    asm = dict(atoms_per_core=atoms_per_core, bbounds=bbounds, per=per)
    return cfg, in_maps, asm


# ============================== device program ==============================

def _build(cfg, stage=0):
    C_t = list(cfg["C_t"])
    SUMC = cfg["SUMC"]
    SC_ALL = cfg["SC_ALL"]
    SC8 = cfg["SC8"]
    offs = np.concatenate([[0], np.cumsum(C_t)]).astype(int)
    CMAX = max(C_t)

    nc = bacc.Bacc("TRN2", target_bir_lowering=False, debug=False,
                   enable_asserts=False, num_devices=NCORES)

    def din(name, shape, dtype=F32):
        return nc.dram_tensor(name, list(shape), dtype, kind="ExternalInput").ap()

    oh75 = din("oh75", [75, APAD])
    E75 = din("E75", [75, 128])
    mmean = din("mmean", [128, TA * BPAD])
    esrc = din("esrc", [128, SUMC * 8], I16)
    edst = din("edst", [128, SUMC])
    exyz = din("exyz", [128, SUMC * 6])
    axyz = din("axyz", [128, TA * 6])
    ncce = din("ncce", [128, 32])
    ncca = din("ncca", [128, 32])
    add32 = din("add32", [128, 32])
    iota = din("iota", [128, 128])
    halfpi = din("halfpi", [128, 1])
    ident = din("ident", [128, 128])
    W = {}
    for l in range(N_CONV):
        for pre in ("m", "c"):
            W[f"{pre}W1_{l}"] = din(f"{pre}W1_{l}", [128, 128])
            W[f"{pre}b1_{l}"] = din(f"{pre}b1_{l}", [128, 1])
            W[f"{pre}W2_{l}"] = din(f"{pre}W2_{l}", [128, 128])
            W[f"{pre}b2_{l}"] = din(f"{pre}b2_{l}", [128, 128])
            W[f"{pre}Wd_{l}"] = din(f"{pre}Wd_{l}", [32, 128])

    Hout = nc.dram_tensor("Hout", [128, BPAD], F32, kind="ExternalOutput").ap()
    hout = nc.dram_tensor("hout", [128, APAD], F32, kind="ExternalOutput").ap()
    if stage == 5:
        dbg = {n: nc.dram_tensor(n, [128, CMAX * 128], F32,
                                 kind="ExternalOutput").ap()
               for n in ("d_gat", "d_S", "d_msg", "d_ws", "d_rws", "d_hacc",
                         "d_phi")}
        dbg["d_aug"] = nc.dram_tensor("d_aug", [128, SC8 * 32], F32,
                                      kind="ExternalOutput").ap()

    AF = mybir.ActivationFunctionType
    OP = mybir.AluOpType

    with tile.TileContext(nc) as tc:
        with tc.tile_pool(name="cst", bufs=1) as cp, \
             tc.tile_pool(name="wrk", bufs=2) as wp, \
             tc.tile_pool(name="psA", bufs=3, space="PSUM") as psA, \
             tc.tile_pool(name="psB", bufs=4, space="PSUM") as psB, \
             tc.tile_pool(name="psH", bufs=1, space="PSUM") as psH, \
             tc.tile_pool(name="drm", bufs=1, space="DRAM") as dp:

            # ---------- load persistent constants ----------
            def load(pool, ap, shape, dtype=F32, name=None):
                t = pool.tile(list(shape), dtype, name=name)
                nc.sync.dma_start(t[:], ap[:])
                return t

            oh_sb = load(cp, oh75, [75, APAD], name="oh_sb")
            e75_sb = load(cp, E75, [75, 128], name="e75_sb")
            mm_sb = load(cp, mmean, [128, TA * BPAD], name="mm_sb")
            esrc_sb = load(cp, esrc, [128, SUMC * 8], I16, name="esrc_sb")
            edst_sb = load(cp, edst, [128, SUMC], name="edst_sb")
            iota_sb = load(cp, iota, [128, 128], name="iota_sb")
            hpi_sb = load(cp, halfpi, [128, 1], name="hpi_sb")
            id_sb = load(cp, ident, [128, 128], name="id_sb")
            w_sb = {k: load(cp, W[k], W[k].shape, name=f"w_{k}") for k in W}

            dist = cp.tile([128, SC_ALL], F32, name="dist")
            env = cp.tile([128, SC_ALL], F32, name="env")
            eod = cp.tile([128, SC_ALL], F32, name="eod")
            aug = cp.tile([128, SC8 * 32], F32, name="aug")
            rbfTa = cp.tile([32, TA * 128], F32, name="rbfTa")
            hT = cp.tile([128, APAD], F32, name="hT")

            # ---------- rbf prep (layer independent), scoped scratch ----------
            with tc.tile_pool(name="prep", bufs=1) as pp:
                exyz_sb = load(pp, exyz, [128, SUMC * 6], name="exyz_sb")
                axyz_sb = load(pp, axyz, [128, TA * 6], name="axyz_sb")
                ne_sb = load(pp, ncce, [128, 32], name="ne_sb")
                na_sb = load(pp, ncca, [128, 32], name="na_sb")
                ad_sb = load(pp, add32, [128, 32], name="ad_sb")

                scr3 = pp.tile([128, SC_ALL * 3], F32, name="scr3")
                recip = pp.tile([128, SC_ALL], F32, name="recip")
                msk = pp.tile([128, SC_ALL], F32, name="msk")
                for (xyz_t, n_sc, o_sc) in ((exyz_sb, SUMC, 0), (axyz_sb, TA, SUMC)):
                    xv = xyz_t[:].rearrange("p (s x) -> p s x", x=6)
                    df = scr3[:, o_sc * 3:(o_sc + n_sc) * 3].rearrange(
                        "p (s x) -> p s x", x=3)
                    nc.vector.tensor_tensor(out=df, in0=xv[:, :, 0:3],
                                            in1=xv[:, :, 3:6], op=OP.subtract)
                    nc.vector.tensor_tensor(out=df, in0=df, in1=df, op=OP.mult)
                    nc.vector.tensor_reduce(out=dist[:, o_sc:o_sc + n_sc], in_=df,
                                            axis=mybir.AxisListType.X, op=OP.add)
                # dist currently holds d^2; sqrt via bit trick + 2x Heron
                # (ACT Sqrt lives in a different act table than Sin/Silu)
                qi = pp.tile([128, SC_ALL], mybir.dt.int32, name="qi")
                rr = pp.tile([128, SC_ALL], F32, name="rr")
                nc.vector.tensor_copy(qi[:], dist[:].bitcast(mybir.dt.int32))
                nc.vector.tensor_scalar(out=qi[:], in0=qi[:], scalar1=1,
                                        scalar2=None, op0=OP.arith_shift_right)
                nc.vector.tensor_scalar(out=qi[:], in0=qi[:], scalar1=0x1fbd1df5,
                                        scalar2=None, op0=OP.add)
                nc.vector.tensor_copy(eod[:], qi[:].bitcast(F32))
                for _ in range(2):
                    nc.vector.reciprocal(rr[:], eod[:])
                    nc.vector.tensor_tensor(out=rr[:], in0=dist[:], in1=rr[:],
                                            op=OP.mult)
                    nc.vector.tensor_add(eod[:], eod[:], rr[:])
                    nc.vector.tensor_scalar(out=eod[:], in0=eod[:], scalar1=0.5,
                                            scalar2=None, op0=OP.mult)
                nc.vector.tensor_copy(dist[:], eod[:])
                nc.vector.reciprocal(recip[:], dist[:])
                for (n_sc, o_sc, cut) in ((SUMC, 0, CUTOFF), (TA, SUMC, CG_CUTOFF)):
                    sl = slice(o_sc, o_sc + n_sc)
                    # env = (0.5*cos(pi*d/cut)+0.5) * (d < cut)
                    nc.scalar.activation(env[:, sl], dist[:, sl], AF.Sin,
                                         bias=hpi_sb[:, 0:1], scale=-math.pi / cut)
                    nc.vector.tensor_scalar(out=env[:, sl], in0=env[:, sl],
                                            scalar1=0.5, scalar2=0.5,
                                            op0=OP.mult, op1=OP.add)
                    nc.vector.tensor_scalar(out=msk[:, sl], in0=dist[:, sl],
                                            scalar1=float(cut), scalar2=None,
                                            op0=OP.is_lt)
                nc.vector.tensor_tensor(out=env[:], in0=env[:], in1=msk[:],
                                        op=OP.mult)
                nc.vector.tensor_tensor(out=eod[:], in0=env[:], in1=recip[:],
                                        op=OP.mult)

                # aug32[:, s, 0:16] = sin(n*pi*d/cut)*env/d, [.., 16] = env
                t32 = pp.tile([128, SC8 * 32], F32, name="t32")
                nc.vector.memset(t32[:], 0.0)
                nc.vector.memset(aug[:], 0.0)
                for (n_sc, o_sc, nt) in ((SUMC, 0, ne_sb), (TA, SUMC, na_sb)):
                    sl3 = t32[:, o_sc * 32:(o_sc + n_sc) * 32].rearrange(
                        "p (s r) -> p s r", r=32)
                    av = aug[:, o_sc * 32:(o_sc + n_sc) * 32].rearrange(
                        "p (s r) -> p s r", r=32)
                    d_b = dist[:, o_sc:o_sc + n_sc].to_broadcast([128, n_sc, 32])
                    nb = nt[:].rearrange("p (a r) -> p a r", a=1) \
                        .to_broadcast([128, n_sc, 32])
                    ab = ad_sb[:].rearrange("p (a r) -> p a r", a=1) \
                        .to_broadcast([128, n_sc, 32])
                    nc.vector.tensor_tensor(out=sl3, in0=nb, in1=d_b, op=OP.mult)
                    nc.vector.tensor_tensor(out=av, in0=ab, in1=d_b, op=OP.mult)
                two_pi = 2.0 * math.pi
                kk = pp.tile([128, SC8 * 32], F32, name="kk")
                nc.vector.tensor_scalar(out=kk[:], in0=t32[:],
                                        scalar1=1.0 / two_pi, scalar2=None,
                                        op0=OP.mult)
                nc.vector.tensor_copy(kk[:].bitcast(mybir.dt.int32), kk[:])
                nc.vector.tensor_copy(kk[:], kk[:].bitcast(mybir.dt.int32))
                nc.vector.tensor_scalar(out=kk[:], in0=kk[:], scalar1=two_pi,
                                        scalar2=None, op0=OP.mult)
                nc.vector.tensor_sub(t32[:], t32[:], kk[:])
                nc.scalar.activation(t32[:], t32[:], AF.Sin)
                nc.vector.tensor_add(t32[:], t32[:], aug[:])
                nc.vector.tensor_tensor(
                    out=aug[:, :SC_ALL * 32].rearrange("p (s r) -> p s r", r=32),
                    in0=t32[:, :SC_ALL * 32].rearrange("p (s r) -> p s r", r=32),
                    in1=eod[:].to_broadcast([128, SC_ALL, 32]),
                    op=OP.mult)

            # atom-chunk rbf transposes (reused both layers), all base 0
            for b0 in range(0, TA, 4):
                bn = min(4, TA - b0)
                ptw = psA.tile([32, 512], F32, name="ptw", tag="psa")
                for k in range(bn):
                    sck = SUMC + b0 + k
                    nc.tensor.transpose(out=ptw[:, k * 128:(k + 1) * 128],
                                        in_=aug[:, sck * 32:(sck + 1) * 32],
                                        identity=id_sb[:])
                nc.vector.tensor_copy(rbfTa[:, b0 * 128:(b0 + bn) * 128],
                                      ptw[:, :bn * 128])

            # ---------- h0 (own atoms): h0_T = E75.T @ onehot75 ----------
            gsz = [512, 512, 512, APAD - 1536]
            go = np.concatenate([[0], np.cumsum(gsz)]).astype(int)
            for g in range(4):
                ph = psA.tile([128, 512], F32, name="ph", tag="psa")
                nc.tensor.matmul(ph[:, :gsz[g]], lhsT=e75_sb[:],
                                 rhs=oh_sb[:, go[g]:go[g + 1]],
                                 start=True, stop=True)
                nc.scalar.copy(hT[:, go[g]:go[g + 1]], ph[:, :gsz[g]])

            # ---------- per layer ----------
            HP = psH.tile([128, BPAD], F32, name="HP")

            def _emit_hcg(l):
                if l == 0:
                    # H0 = scatter_mean(h1): transpose hT tiles to rows
                    for t in range(TA):
                        ptr2 = psB.tile([128, 128], F32, name="ptr2", tag="psb")
                        nc.tensor.transpose(out=ptr2[:],
                                            in_=hT[:, t * 128:(t + 1) * 128],
                                            identity=id_sb[:])
                        hr = wp.tile([128, 128], F32, name="hr", tag="hr")
                        nc.scalar.copy(hr[:], ptr2[:])
                        nc.tensor.matmul(
                            HP[:], lhsT=hr[:],
                            rhs=mm_sb[:, t * BPAD:(t + 1) * BPAD],
                            start=(t == 0), stop=False)

                # cg path: c0 = (silu(h@cW1+cb1)@cW2[:, :128]+cb2) * w_c
                z1c = wp.tile([128, APAD], F32, name=f"z1c_{l}", tag="z1c", bufs=1)
                for g in range(4):
                    pz = psA.tile([128, 512], F32, name="pzc", tag="psa")
                    nc.tensor.matmul(pz[:, :gsz[g]], lhsT=w_sb[f"cW1_{l}"][:],
                                     rhs=hT[:, go[g]:go[g + 1]],
                                     start=True, stop=True)
                    nc.scalar.activation(z1c[:, go[g]:go[g + 1]], pz[:, :gsz[g]],
                                         AF.Silu, bias=w_sb[f"cb1_{l}"][:, 0:1])
                for t in range(TA):
                    ppc = psB.tile([128, 128], F32, name="ppc", tag="psb")
                    nc.tensor.matmul(ppc[:], lhsT=z1c[:, t * 128:(t + 1) * 128],
                                     rhs=w_sb[f"cW2_{l}"][:],
                                     start=True, stop=True)
                    phc = wp.tile([128, 128], F32, name="phc", tag="phc")
                    nc.vector.tensor_add(phc[:], ppc[:], w_sb[f"cb2_{l}"][:])
                    pwc = psB.tile([128, 128], F32, name="pwc", tag="psb")
                    nc.tensor.matmul(pwc[:], lhsT=rbfTa[:, t * 128:(t + 1) * 128],
                                     rhs=w_sb[f"cWd_{l}"][:],
                                     start=True, stop=True)
                    c0 = wp.tile([128, 128], F32, name="c0", tag="c0")
                    nc.vector.tensor_tensor(out=c0[:], in0=phc[:], in1=pwc[:],
                                            op=OP.mult)
                    nc.tensor.matmul(
                        HP[:], lhsT=c0[:],
                        rhs=mm_sb[:, t * BPAD:(t + 1) * BPAD],
                        start=False,
                        stop=(l == N_CONV - 1 and t == TA - 1))

            for l in range(N_CONV if stage in (0, 4, 5) else (stage >= 2) * N_CONV):
                bounce = dp.tile([APAD, 128], F32, name=f"bounce{l}")
                phiall = dp.tile([NPAD, 128], F32, name=f"phiall{l}",
                                 addr_space="Shared")

                # z1 = silu(W1.T @ hT + b1)  (msg MLP, own atoms)
                z1 = wp.tile([128, APAD], F32, name=f"z1_{l}", tag="z1", bufs=1)
                for g in range(4):
                    pz = psA.tile([128, 512], F32, name="pz", tag="psa")
                    nc.tensor.matmul(pz[:, :gsz[g]], lhsT=w_sb[f"mW1_{l}"][:],
                                     rhs=hT[:, go[g]:go[g + 1]],
                                     start=True, stop=True)
                    nc.scalar.activation(z1[:, go[g]:go[g + 1]], pz[:, :gsz[g]],
                                         AF.Silu, bias=w_sb[f"mb1_{l}"][:, 0:1])
                # phi rows per atom tile -> bounce DRAM
                for t in range(TA):
                    pp2 = psB.tile([128, 128], F32, name="pp2", tag="psb")
                    nc.tensor.matmul(pp2[:], lhsT=z1[:, t * 128:(t + 1) * 128],
                                     rhs=w_sb[f"mW2_{l}"][:],
                                     start=True, stop=True)
                    pr = wp.tile([128, 128], F32, name="pr", tag="phir", bufs=3)
                    nc.vector.tensor_add(pr[:], pp2[:], w_sb[f"mb2_{l}"][:])
                    nc.sync.dma_start(bounce[t * 128:(t + 1) * 128, :], pr[:])
                    if stage == 5 and l == 0 and t == 0:
                        nc.sync.dma_start(dbg["d_phi"][:, :128], pr[:])

                nc.gpsimd.collective_compute(
                    "AllGather", OP.bypass,
                    replica_groups=[list(range(NCORES))],
                    ins=[bounce[:]], outs=[phiall[:]])

                # previous layer's H/scatter-mean + cg work overlaps this AG
                if l > 0 and stage in (0, 4):
                    _emit_hcg(l - 1)

                # ---------- edge stage ----------
                for t in range(TA if stage in (0, 3, 4, 5) else 0):
                    Ct = C_t[t]
                    gat = wp.tile([128, CMAX * 128], F32, name="gat", tag="gat", bufs=3)
                    for a0 in range(0, Ct, 8):  # dma_gather caps at 1024 idxs
                        an = min(8, Ct - a0)
                        g3 = gat[:, (a0) * 128:(a0 + an) * 128].rearrange(
                            "p (c f) -> p c f", f=128)
                        nc.gpsimd.dma_gather(
                            g3, phiall[:],
                            esrc_sb[:, (offs[t] + a0) * 8:(offs[t] + a0 + an) * 8],
                            an * 128, an * 128, 128)
                    S = wp.tile([128, CMAX * 128], F32, name="S", tag="S", bufs=3)
                    nc.vector.tensor_tensor(
                        out=S[:, :Ct * 128].rearrange("p (c d) -> p c d", d=128),
                        in0=iota_sb[:].rearrange("p (a d) -> p a d", a=1)
                            .to_broadcast([128, Ct, 128]),
                        in1=edst_sb[:, offs[t]:offs[t] + Ct]
                            .to_broadcast([128, Ct, 128]),
                        op=OP.is_equal)
                    rws = wp.tile([32, CMAX * 128], F32, name="rws", tag="rws", bufs=3)
                    for b0 in range(0, Ct, 4):
                        bn = min(4, Ct - b0)
                        ptw = psA.tile([32, 512], F32, name="ptw2", tag="psa")
                        for k in range(bn):
                            sck = offs[t] + b0 + k
                            nc.tensor.transpose(
                                out=ptw[:, k * 128:(k + 1) * 128],
                                in_=aug[:, sck * 32:(sck + 1) * 32],
                                identity=id_sb[:])
                        nc.vector.tensor_copy(rws[:, b0 * 128:(b0 + bn) * 128],
                                              ptw[:, :bn * 128])
                    hacc = psB.tile([128, 128], F32, name="hacc", tag="psb")
                    nq = (Ct + 3) // 4
                    for q in range(nq):
                        qn = min(4, Ct - q * 4)
                        pws = psA.tile([128, 512], F32, name="pws", tag="psa")
                        for gg in range(qn):
                            g = q * 4 + gg
                            nc.tensor.matmul(
                                pws[:, gg * 128:(gg + 1) * 128],
                                lhsT=rws[:, g * 128:(g + 1) * 128],
                                rhs=w_sb[f"mWd_{l}"][:],
                                start=True, stop=True)
                        msg = wp.tile([128, 512], F32, name="msg", tag="msg", bufs=4)
                        nc.vector.tensor_tensor(
                            out=msg[:, :qn * 128],
                            in0=gat[:, q * 512:q * 512 + qn * 128],
                            in1=pws[:, :qn * 128], op=OP.mult)
                        if stage == 5 and l == 0 and t == 0:
                            ws_sb = wp.tile([128, 512], F32, name="ws_sb",
                                            tag="ws_sb")
                            nc.vector.tensor_copy(ws_sb[:, :qn * 128],
                                                  pws[:, :qn * 128])
                            nc.sync.dma_start(
                                dbg["d_ws"][:, q * 512:q * 512 + qn * 128],
                                ws_sb[:, :qn * 128])
                            nc.sync.dma_start(
                                dbg["d_msg"][:, q * 512:q * 512 + qn * 128],
                                msg[:, :qn * 128])
                        for gg in range(qn):
                            g = q * 4 + gg
                            nc.tensor.matmul(
                                hacc[:],
                                lhsT=msg[:, gg * 128:(gg + 1) * 128],
                                rhs=S[:, g * 128:(g + 1) * 128],
                                start=(g == 0), stop=(g == Ct - 1))
                    if stage == 5 and l == 0 and t == 0:
                        nc.sync.dma_start(dbg["d_gat"][:, :Ct * 128],
                                          gat[:, :Ct * 128])
                        nc.sync.dma_start(dbg["d_S"][:, :Ct * 128],
                                          S[:, :Ct * 128])
                        nc.sync.dma_start(dbg["d_rws"][:32, :Ct * 128],
                                          rws[:, :Ct * 128])
                        hac_sb = cp.tile([128, 128], F32, name="hac_sb")
                        nc.vector.tensor_copy(hac_sb[:], hacc[:])
                        nc.sync.dma_start(dbg["d_hacc"][:, :128], hac_sb[:])
                        nc.sync.dma_start(dbg["d_aug"][:, :SC_ALL * 32],
                                          aug[:, :SC_ALL * 32])
                    nc.vector.tensor_add(out=hT[:, t * 128:(t + 1) * 128],
                                         in0=hT[:, t * 128:(t + 1) * 128],
                                         in1=hacc[:])

            if stage in (0, 4):
                _emit_hcg(N_CONV - 1)

            # ---------- outputs ----------
            Hs = cp.tile([128, BPAD], F32, name="Hs")
            if stage in (0, 4):
                nc.vector.tensor_copy(Hs[:], HP[:])
            else:
                nc.vector.memset(Hs[:], 0.0)
            nc.sync.dma_start(Hout[:], Hs[:])
            nc.sync.dma_start(hout[:], hT[:])

    nc.compile()
    return nc


# ================================= runner ==================================

def kernel(z, xyz, cg_z, cg_xyz, mapping, nbr_list, cg_nbr_list,
           atom_emb, res_emb,
           msg_W1, msg_b1, msg_W2, msg_b2, msg_Wd, msg_bd,
           cg_W1, cg_b1, cg_W2, cg_b2, cg_Wd, cg_bd):
    global LAST_RESULT
    cfg, in_maps, asm = _host_prep(
        z, xyz, cg_z, cg_xyz, mapping, nbr_list, atom_emb, res_emb,
        msg_W1, msg_b1, msg_W2, msg_b2, msg_Wd, msg_bd,
        cg_W1, cg_b1, cg_W2, cg_b2, cg_Wd, cg_bd)

    stage = int(os.environ.get("BASSK_STAGE", "0"))
    key = (cfg["C_t"], stage)
    if key not in _CACHE:
        _CACHE[key] = _build(cfg, stage)
    nc = _CACHE[key]

    trace = bool(int(os.environ.get("BASSK_TRACE", "0")))
    if trace:
        try:
            res = bass_utils.run_bass_kernel_spmd(
                nc, in_maps, core_ids=list(range(NCORES)), trace=True)
        except Exception:
            trace = False
    if not trace:
        res = bass_utils.run_bass_kernel_spmd(
            nc, in_maps, core_ids=list(range(NCORES)))
    LAST_RESULT = res
    global _LAST_NC, _LAST_INMAPS
    _LAST_NC, _LAST_INMAPS = nc, in_maps

    # ---- assemble outputs
    H = np.zeros((N_CG, FEAT), dtype=np.float32)
    h = np.zeros((N_ATOMS, FEAT), dtype=np.float32)
    bbounds = asm["bbounds"]
    for c in range(NCORES):
        r = res.results[c]
        nb = asm["per"][c]
        H[bbounds[c]:bbounds[c + 1]] = r["Hout"].T[:nb]
        a_c = asm["atoms_per_core"][c]
        h[a_c] = r["hout"].T[:len(a_c)]
    return H, h
